# revision 7
# baseline (speedup 1.0000x reference)
"""Multi-head GQA attention (B=2, S=2048, H=4096, 32 q-heads / 8 kv-heads,
HD=128, rotary, causal) on 8 TRN2 NeuronCores.

Sharding: tensor-parallel over heads, 8-way — core c owns q-heads
[4c, 4c+4) and kv-head c; wq/wk/wv column-sharded, wo row-sharded.  Each
core computes a partial wo product over its head slice for both batches;
the host sums the 8 partials (the TP reduction) and transposes back.

All on-device dataflow is in transposed layout [feature, seq] so no
device-side transposes are needed; the host pre-transposes x and the
weight shards during sharding.  Rotary pairs are laid out so the (x0,x1)
pair swap is a 32-partition-quadrant stream_shuffle on the Vector engine.
"""
import sys

if "/opt/trn_rl_repo" not in sys.path:
    sys.path.insert(0, "/opt/trn_rl_repo")

import numpy as np
import ml_dtypes

from concourse import bacc, tile, mybir, bass_isa
from concourse.bass_utils import run_bass_kernel_spmd

F32 = mybir.dt.float32
F32R = mybir.dt.float32r
BF16 = mybir.dt.bfloat16
EXP = mybir.ActivationFunctionType.Exp
COPY = mybir.ActivationFunctionType.Copy
BF16NP = ml_dtypes.bfloat16

B, S, H = 2, 2048, 4096
NH, NKV, HD = 32, 8, 128
NCORES = 8
QH = NH // NCORES          # 4 q-heads per core
RQ = QH * HD               # 512 q rows per core
SB = 512                   # projection seq block
NSB = S // SB              # 4
IB = 512                   # attention i block
NIB = S // IB              # 4
HC = H // 128              # 32 contraction chunks
NJT = S // 128             # 16 j tiles

# stream_shuffle mask: swap 16-partition halves within each 32-partition quadrant
SHUF = list(range(16, 32)) + list(range(16))

LAST_EXEC_NS = None
_CACHED = None


def _build(dbg=False):
    nc = bacc.Bacc("TRN2", target_bir_lowering=False, debug=False,
                   num_devices=NCORES)

    xt_e = nc.dram_tensor("xt", [B, H, S], BF16, kind="ExternalInput")
    cc_e = nc.dram_tensor("cc", [B, 128, S], F32, kind="ExternalInput")
    ss_e = nc.dram_tensor("ss2", [B, 128, S], F32, kind="ExternalInput")
    wq_e = nc.dram_tensor("wqt", [H, RQ], BF16, kind="ExternalInput")
    wk_e = nc.dram_tensor("wkt", [H, HD], BF16, kind="ExternalInput")
    wv_e = nc.dram_tensor("wvt", [H, HD], BF16, kind="ExternalInput")
    wo_e = nc.dram_tensor("wot", [RQ, H], BF16, kind="ExternalInput")
    mk_e = nc.dram_tensor("masks", [4, 128, IB], BF16, kind="ExternalInput")
    id_e = nc.dram_tensor("ident", [128, 128], BF16, kind="ExternalInput")
    out_e = nc.dram_tensor("out", [B, H, S], BF16, kind="ExternalOutput")
    if dbg:
        dq_e = nc.dram_tensor("dbg_qt", [128, QH, S], F32R, kind="ExternalOutput")
        dk_e = nc.dram_tensor("dbg_kt", [128, S], F32R, kind="ExternalOutput")
        dv_e = nc.dram_tensor("dbg_vn", [128, NJT, HD], BF16, kind="ExternalOutput")
        da_e = nc.dram_tensor("dbg_at", [NIB, 128, QH, IB], BF16, kind="ExternalOutput")

    with tile.TileContext(nc) as tc:
        with (nc.allow_low_precision(reason="bf16/f32r compute by design"),
              tc.tile_pool(name="wpool", bufs=1) as wp,
              tc.tile_pool(name="state", bufs=1) as st,
              tc.tile_pool(name="att", bufs=2) as ap_,
              tc.tile_pool(name="xin", bufs=3) as xp,
              tc.tile_pool(name="probs", bufs=5) as pp,
              tc.tile_pool(name="rot", bufs=2) as rp,
              tc.tile_pool(name="stage", bufs=2) as sg,
              tc.tile_pool(name="small", bufs=4) as sm_p,
              tc.tile_pool(name="ps", bufs=6, space="PSUM") as ps):

            # ---- resident weights ----
            wq_s = wp.tile([128, HC, RQ], BF16)
            wk_s = wp.tile([128, HC, HD], BF16)
            wv_s = wp.tile([128, HC, HD], BF16)
            wo_s = wp.tile([128, QH, H], BF16)
            mk_s = wp.tile([128, 4, IB], BF16)
            id_s = wp.tile([128, 128], BF16)

            # weight chunks are DMA'd just-in-time, interleaved with the
            # first projection block's x loads so the PE starts immediately
            _wchunk_loaded = [False] * (HC // 4)
            _tail_loaded = [False]

            def load_wchunk(hg):
                # one grouped DMA per 4 contraction chunks
                if _wchunk_loaded[hg]:
                    return
                _wchunk_loaded[hg] = True
                r0, r1 = hg * 512, (hg + 1) * 512
                nc.sync.dma_start(
                    out=wq_s[:, 4 * hg:4 * hg + 4],
                    in_=wq_e.ap()[r0:r1].rearrange("(c p) r -> p c r", c=4))
                nc.sync.dma_start(
                    out=wk_s[:, 4 * hg:4 * hg + 4],
                    in_=wk_e.ap()[r0:r1].rearrange("(c p) r -> p c r", c=4))
                nc.sync.dma_start(
                    out=wv_s[:, 4 * hg:4 * hg + 4],
                    in_=wv_e.ap()[r0:r1].rearrange("(c p) r -> p c r", c=4))

            def load_tail_weights():
                if _tail_loaded[0]:
                    return
                _tail_loaded[0] = True
                nc.sync.dma_start(out=id_s[:], in_=id_e.ap())
                for k in range(4):
                    nc.sync.dma_start(out=mk_s[:, k], in_=mk_e.ap()[k])
                for rc in range(QH):
                    nc.sync.dma_start(out=wo_s[:, rc], in_=wo_e.ap()[rc * 128:(rc + 1) * 128])

            _rot_n = [0]

            def rotary(src_ps, s0, ccb, ssb, dst_ap):
                # dst = src*CC + shuffle(src)*SS2, written as f32r
                r = _rot_n[0] = (_rot_n[0] + 1) % 2
                qs = rp.tile([128, SB], F32, tag=f"qs{r}", bufs=1)
                nc.scalar.activation(qs[:], src_ps[:], COPY)
                qw = rp.tile([128, SB], F32, tag=f"qw{r}", bufs=1)
                nc.vector.stream_shuffle(qw[:], qs[:], SHUF)
                nc.vector.tensor_mul(qs[:], qs[:], ccb[:, s0:s0 + SB])
                nc.vector.tensor_mul(qw[:], qw[:], ssb[:, s0:s0 + SB])
                nc.vector.tensor_add(dst_ap, qs[:], qw[:])

            pending_wo = []

            def emit_wo(wb, wib, wat):
                wi0 = wib * IB
                for ho in range(H // 128):
                    acc = ps.tile([128, IB], F32, tag="ps", name="acc")
                    for rc in range(QH):
                        nc.tensor.matmul(acc[:],
                                         wo_s[:, rc, ho * 128:(ho + 1) * 128],
                                         wat[:, rc],
                                         start=(rc == 0), stop=(rc == QH - 1))
                    og = sg.tile([128, IB], BF16, tag=f"og{ho % 3}", name="og",
                                 bufs=1)
                    if ho % 2 == 0:
                        nc.scalar.activation(og[:], acc[:], COPY)
                    else:
                        nc.vector.tensor_copy(og[:], acc[:])
                    oeng = nc.scalar if ho % 2 == 0 else nc.sync
                    oeng.dma_start(
                        out=out_e.ap()[wb, ho * 128:(ho + 1) * 128, wi0:wi0 + IB],
                        in_=og[:])

            for b in range(B):
                qT = st.tile([128, QH, S], F32R, tag="qT")
                kT = st.tile([128, S], F32R, tag="kT")
                vn = st.tile([128, NJT, HD], BF16, tag="vn")
                ccb = st.tile([128, S], F32, tag="cc")
                ssb = st.tile([128, S], F32, tag="ss")
                nc.scalar.dma_start(out=ccb[:], in_=cc_e.ap()[b])
                nc.scalar.dma_start(out=ssb[:], in_=ss_e.ap()[b])

                # interleaved: projection block sb feeds attention block
                # ib == sb immediately (causal: ib needs only j <= (sb+1)*SB)
                for sb in range(NSB):
                    s0 = sb * SB
                    # ---- projection + rotary for seq block sb ----
                    qa = [ps.tile([128, SB], F32, tag="ps", name=f"qa{rc}")
                          for rc in range(QH)]
                    ka = ps.tile([128, SB], F32, tag="ps")
                    vta = ps.tile([128, SB], F32, tag="ps")
                    for hg in range(HC // 4):
                        load_wchunk(hg)
                        xt_g = xp.tile([128, 4, SB], BF16, tag=f"xt{hg % 4}",
                                       bufs=1)
                        eng = nc.sync if hg % 2 == 0 else nc.scalar
                        eng.dma_start(
                            out=xt_g[:],
                            in_=xt_e.ap()[b, hg * 512:(hg + 1) * 512, s0:s0 + SB]
                            .rearrange("(c p) s -> p c s", c=4))
                        for c in range(4):
                            hc = hg * 4 + c
                            st_, sp_ = (hc == 0), (hc == HC - 1)
                            for rc in range(QH):
                                nc.tensor.matmul(qa[rc][:],
                                                 wq_s[:, hc, rc * 128:(rc + 1) * 128],
                                                 xt_g[:, c], start=st_, stop=sp_)
                            nc.tensor.matmul(ka[:], wk_s[:, hc], xt_g[:, c],
                                             start=st_, stop=sp_)
                            nc.tensor.matmul(vta[:], wv_s[:, hc], xt_g[:, c],
                                             start=st_, stop=sp_)
                    load_tail_weights()
                    rotary(ka, s0, ccb, ssb, kT[:, s0:s0 + SB])
                    for rc in range(QH):
                        rotary(qa[rc], s0, ccb, ssb, qT[:, rc, s0:s0 + SB])

                    # previous block's wo matmuls go here: independent PE
                    # work that covers the rotary-chain latency on ACT/DVE
                    # and the vta->vt_s copy before the transposes
                    if pending_wo:
                        emit_wo(*pending_wo.pop())

                    # vT [d, s] -> natural v j-tiles via PE transpose (bf16)
                    vt_s = rp.tile([128, SB], BF16, tag="vts")
                    nc.scalar.activation(vt_s[:], vta[:], COPY)
                    for t in range(SB // 128):
                        tp = ps.tile([128, 128], BF16, tag="ps", name="tp")
                        nc.tensor.transpose(tp[:], vt_s[:, t * 128:(t + 1) * 128],
                                            id_s[:])
                        nc.scalar.activation(vn[:, (SB // 128) * sb + t], tp[:], COPY)

                    if dbg and b == 0 and sb == NSB - 1:
                        nc.sync.dma_start(out=dq_e.ap(), in_=qT[:])
                        nc.sync.dma_start(out=dk_e.ap(), in_=kT[:])
                        nc.sync.dma_start(out=dv_e.ap(), in_=vn[:])

                    # ---- attention for i block ib == sb ----
                    ib = sb
                    i0 = ib * IB
                    njt = (ib + 1) * (IB // 128)
                    at = ap_.tile([128, QH, IB], BF16, tag="at")
                    pvf = []
                    sums = []
                    for h in range(QH):
                        pv = ps.tile([128, IB], F32, tag="ps")
                        psm = sm_p.tile([128, IB], BF16, tag="psm", name=f"psm{h}")
                        pend = []
                        def flush(stop):
                            pjt, ppb = pend.pop(0)
                            nc.tensor.matmul(pv[:], vn[:, pjt], ppb[:],
                                             start=(pjt == 0), stop=stop)
                        for jt in range(njt):
                            sc = ps.tile([128, IB], F32, tag=f"sc{jt % 2}",
                                         name="sc", bufs=1)
                            nc.tensor.matmul(sc[:], kT[:, jt * 128:(jt + 1) * 128],
                                             qT[:, h, i0:i0 + IB],
                                             start=True, stop=True)
                            pb = pp.tile([128, IB], BF16, tag=f"pb{jt % 5}",
                                         bufs=1)
                            nc.scalar.activation(pb[:], sc[:], EXP)
                            kd = jt - ib * 4
                            if kd >= 0:
                                nc.vector.tensor_mul(pb[:], pb[:], mk_s[:, kd])
                            # probs running sum on DVE (replaces the PE
                            # ones-matmul row-sums, ~117us of PE time)
                            if jt == 0:
                                nc.vector.tensor_copy(psm[:], pb[:])
                            else:
                                nc.vector.tensor_add(psm[:], psm[:], pb[:])
                            pend.append((jt, pb))
                            if len(pend) > 2:
                                flush(False)
                        while pend:
                            flush(stop=(len(pend) == 1))
                        # stash PV; normalization deferred to the loop below
                        pvh = sg.tile([128, IB], F32, tag="pvf", name=f"pvf{h}", bufs=4)
                        nc.scalar.activation(pvh[:], pv[:], COPY)
                        pvf.append(pvh)
                        sums.append(psm)
                    for h in range(QH):
                        # softmax denominator: cross-partition sum broadcast
                        # to all partitions on the (otherwise idle) GpSimd
                        den = sg.tile([128, IB], F32, tag=f"den{h % 2}", bufs=1)
                        nc.gpsimd.partition_all_reduce(den[:], sums[h][:], 128,
                                                       bass_isa.ReduceOp.add)
                        rb = sg.tile([128, IB], F32, tag=f"rb{h % 2}", bufs=1)
                        nc.vector.reciprocal_approx_fast(rb[:], den[:])
                        nc.gpsimd.tensor_mul(at[:, h], pvf[h][:], rb[:])
                    if dbg and b == 0:
                        nc.sync.dma_start(out=da_e.ap()[ib], in_=at[:])
                    pending_wo.append((b, ib, at))

            emit_wo(*pending_wo.pop())

    nc.compile()
    return nc


def _prep(x, freqs_cos, freqs_sin, wq, wk, wv, wo):
    """Shard + pre-transpose inputs for the 8 cores."""
    # rotary pair permutation: within each 32-partition quadrant, x0 of
    # pairs [16q,16q+16) sits in local slots 0..15 and x1 in 16..31.
    perm = np.zeros(HD, dtype=np.int64)
    pair = np.zeros(128, dtype=np.int64)
    sign = np.zeros(128, dtype=np.float32)
    for q in range(4):
        for t in range(16):
            perm[32 * q + t] = 2 * (16 * q + t)
            perm[32 * q + 16 + t] = 2 * (16 * q + t) + 1
            pair[32 * q + t] = 16 * q + t
            pair[32 * q + 16 + t] = 16 * q + t
            sign[32 * q + t] = -1.0
            sign[32 * q + 16 + t] = 1.0

    xt = np.ascontiguousarray(x.transpose(0, 2, 1)).astype(BF16NP)  # [B,H,S]
    cc = np.ascontiguousarray(freqs_cos[:, :, pair].transpose(0, 2, 1)).astype(np.float32)
    ss2 = np.ascontiguousarray((freqs_sin[:, :, pair] * sign).transpose(0, 2, 1)).astype(np.float32)

    # causal masks for the 4 j-tiles inside a diagonal 512-token i block
    masks = np.zeros((4, 128, IB), dtype=BF16NP)
    jj = np.arange(128)[:, None]
    ii = np.arange(IB)[None, :]
    for k in range(4):
        masks[k] = (k * 128 + jj <= ii).astype(BF16NP)

    scale = np.float32(1.0 / np.sqrt(HD))
    in_maps = []
    for c in range(NCORES):
        wq_c = (wq[c * RQ:(c + 1) * RQ] * scale).reshape(QH, HD, H)[:, perm, :]
        wqt = np.ascontiguousarray(wq_c.reshape(RQ, H).T).astype(BF16NP)
        wk_c = wk[c * HD:(c + 1) * HD][perm, :]
        wkt = np.ascontiguousarray(wk_c.T).astype(BF16NP)
        wvt = np.ascontiguousarray(wv[c * HD:(c + 1) * HD].T).astype(BF16NP)
        wot = np.ascontiguousarray(wo[:, c * RQ:(c + 1) * RQ].T).astype(BF16NP)
        in_maps.append({
            "xt": xt, "cc": cc, "ss2": ss2,
            "wqt": wqt, "wkt": wkt, "wvt": wvt, "wot": wot,
            "masks": masks, "ident": np.eye(128, dtype=BF16NP),
        })
    return in_maps


def kernel(x, freqs_cos, freqs_sin, wq, wk, wv, wo):
    global _CACHED, LAST_EXEC_NS
    x = np.asarray(x, dtype=np.float32)
    freqs_cos = np.asarray(freqs_cos, dtype=np.float32)
    freqs_sin = np.asarray(freqs_sin, dtype=np.float32)
    wq = np.asarray(wq, dtype=np.float32)
    wk = np.asarray(wk, dtype=np.float32)
    wv = np.asarray(wv, dtype=np.float32)
    wo = np.asarray(wo, dtype=np.float32)

    if _CACHED is None:
        _CACHED = _build()
    nc = _CACHED

    in_maps = _prep(x, freqs_cos, freqs_sin, wq, wk, wv, wo)
    res = run_bass_kernel_spmd(nc, in_maps, core_ids=list(range(NCORES)))
    LAST_EXEC_NS = res.exec_time_ns

    # unshard: sum the 8 partial wo products, then [B,H,S] -> [B,S,H]
    acc = res.results[0]["out"].astype(np.float64)
    for c in range(1, NCORES):
        acc += res.results[c]["out"]
    return np.ascontiguousarray(acc.transpose(0, 2, 1)).astype(np.float32)



# revision 11
# speedup vs baseline: 1.2685x; 1.2685x over previous
"""Multi-head GQA attention (B=2, S=2048, H=4096, 32 q-heads / 8 kv-heads,
HD=128, rotary, causal) on 8 TRN2 NeuronCores.

Sharding: tensor-parallel over heads, 8-way — core c owns q-heads
[4c, 4c+4) and kv-head c; wq/wk/wv column-sharded, wo row-sharded.  Each
core computes a partial wo product over its head slice for both batches;
the host sums the 8 partials (the TP reduction) and transposes back.

All on-device dataflow is in transposed layout [feature, seq] so no
device-side transposes are needed; the host pre-transposes x and the
weight shards during sharding.  Rotary pairs are laid out so the (x0,x1)
pair swap is a 32-partition-quadrant stream_shuffle on the Vector engine.
"""
import sys

if "/opt/trn_rl_repo" not in sys.path:
    sys.path.insert(0, "/opt/trn_rl_repo")

import numpy as np
import ml_dtypes

from concourse import bacc, tile, mybir, bass_isa
from concourse.bass_utils import run_bass_kernel_spmd

F32 = mybir.dt.float32
F32R = mybir.dt.float32r
BF16 = mybir.dt.bfloat16
EXP = mybir.ActivationFunctionType.Exp
COPY = mybir.ActivationFunctionType.Copy
BF16NP = ml_dtypes.bfloat16

B, S, H = 2, 2048, 4096
NH, NKV, HD = 32, 8, 128
NCORES = 8
QH = NH // NCORES          # 4 q-heads per core
RQ = QH * HD               # 512 q rows per core
SB = 512                   # projection seq block
NSB = S // SB              # 4
IB = 512                   # attention i block
NIB = S // IB              # 4
HC = H // 128              # 32 contraction chunks
NJT = S // 128             # 16 j tiles

# stream_shuffle mask: swap 16-partition halves within each 32-partition quadrant
SHUF = list(range(16, 32)) + list(range(16))

LAST_EXEC_NS = None
_CACHED = None


def _build(dbg=False):
    nc = bacc.Bacc("TRN2", target_bir_lowering=False, debug=False,
                   num_devices=NCORES)

    xt_e = nc.dram_tensor("xt", [B, H, S], BF16, kind="ExternalInput")
    cc_e = nc.dram_tensor("cc", [B, 128, S], F32, kind="ExternalInput")
    ss_e = nc.dram_tensor("ss2", [B, 128, S], F32, kind="ExternalInput")
    wq_e = nc.dram_tensor("wqt", [H, RQ], BF16, kind="ExternalInput")
    wk_e = nc.dram_tensor("wkt", [H, HD], BF16, kind="ExternalInput")
    wv_e = nc.dram_tensor("wvt", [H, HD], BF16, kind="ExternalInput")
    wo_e = nc.dram_tensor("wot", [RQ, H], BF16, kind="ExternalInput")
    mk_e = nc.dram_tensor("masks", [4, 128, IB], BF16, kind="ExternalInput")
    id_e = nc.dram_tensor("ident", [128, 128], BF16, kind="ExternalInput")
    out_e = nc.dram_tensor("out", [B, H, S], BF16, kind="ExternalOutput")
    if dbg:
        dq_e = nc.dram_tensor("dbg_qt", [128, QH, S], BF16, kind="ExternalOutput")
        dk_e = nc.dram_tensor("dbg_kt", [128, S], BF16, kind="ExternalOutput")
        dv_e = nc.dram_tensor("dbg_vn", [128, NJT, HD], BF16, kind="ExternalOutput")
        da_e = nc.dram_tensor("dbg_at", [NIB, 128, QH, IB], BF16, kind="ExternalOutput")

    with tile.TileContext(nc) as tc:
        with (nc.allow_low_precision(reason="bf16/f32r compute by design"),
              tc.tile_pool(name="wpool", bufs=1) as wp,
              tc.tile_pool(name="state", bufs=1) as st,
              tc.tile_pool(name="att", bufs=2) as ap_,
              tc.tile_pool(name="xin", bufs=3) as xp,
              tc.tile_pool(name="probs", bufs=5) as pp,
              tc.tile_pool(name="rot", bufs=2) as rp,
              tc.tile_pool(name="stage", bufs=2) as sg,
              tc.tile_pool(name="small", bufs=4) as sm_p,
              tc.tile_pool(name="ps", bufs=6, space="PSUM") as ps):

            # ---- resident weights ----
            wq_s = wp.tile([128, HC, RQ], BF16)
            wk_s = wp.tile([128, HC, HD], BF16)
            wv_s = wp.tile([128, HC, HD], BF16)
            wo_s = wp.tile([128, QH, H], BF16)
            mk_s = wp.tile([128, 4, IB], BF16)
            id_s = wp.tile([128, 128], BF16)
            ones_b = wp.tile([128, 1], BF16)
            ones1f = wp.tile([1, 128], F32)
            ones1 = wp.tile([1, 128], F32R)
            nc.vector.memset(ones_b[:], 1.0)
            nc.vector.memset(ones1f[:], 1.0)
            nc.vector.tensor_copy(ones1[:], ones1f[:])

            # weight chunks are DMA'd just-in-time, interleaved with the
            # first projection block's x loads so the PE starts immediately
            _wchunk_loaded = [False] * (HC // 4)
            _tail_loaded = [False]

            def load_wchunk(hg):
                # one grouped DMA per 4 contraction chunks
                if _wchunk_loaded[hg]:
                    return
                _wchunk_loaded[hg] = True
                r0, r1 = hg * 512, (hg + 1) * 512
                nc.sync.dma_start(
                    out=wq_s[:, 4 * hg:4 * hg + 4],
                    in_=wq_e.ap()[r0:r1].rearrange("(c p) r -> p c r", c=4))
                nc.sync.dma_start(
                    out=wk_s[:, 4 * hg:4 * hg + 4],
                    in_=wk_e.ap()[r0:r1].rearrange("(c p) r -> p c r", c=4))
                nc.sync.dma_start(
                    out=wv_s[:, 4 * hg:4 * hg + 4],
                    in_=wv_e.ap()[r0:r1].rearrange("(c p) r -> p c r", c=4))

            def load_tail_weights():
                if _tail_loaded[0]:
                    return
                _tail_loaded[0] = True
                nc.sync.dma_start(out=id_s[:], in_=id_e.ap())
                for k in range(4):
                    nc.sync.dma_start(out=mk_s[:, k], in_=mk_e.ap()[k])
                for rc in range(QH):
                    nc.sync.dma_start(out=wo_s[:, rc], in_=wo_e.ap()[rc * 128:(rc + 1) * 128])

            _rot_n = [0]

            def rotary(src_ps, s0, ccb, ssb, dst_ap):
                # dst = src*CC + shuffle(src)*SS2, written as f32r
                r = _rot_n[0] = (_rot_n[0] + 1) % 2
                qs = rp.tile([128, SB], F32, tag=f"qs{r}", bufs=1)
                nc.scalar.activation(qs[:], src_ps[:], COPY)
                qw = rp.tile([128, SB], F32, tag=f"qw{r}", bufs=1)
                nc.vector.stream_shuffle(qw[:], qs[:], SHUF)
                nc.vector.tensor_mul(qs[:], qs[:], ccb[:, s0:s0 + SB])
                nc.vector.tensor_mul(qw[:], qw[:], ssb[:, s0:s0 + SB])
                nc.vector.tensor_add(dst_ap, qs[:], qw[:])

            pending_wo = []

            def emit_wo(wb, wib, wat):
                wi0 = wib * IB
                for ho in range(H // 128):
                    acc = ps.tile([128, IB], F32, tag="ps", name="acc")
                    for rc in range(QH):
                        nc.tensor.matmul(acc[:],
                                         wo_s[:, rc, ho * 128:(ho + 1) * 128],
                                         wat[:, rc],
                                         start=(rc == 0), stop=(rc == QH - 1))
                    og = sg.tile([128, IB], BF16, tag=f"og{ho % 3}", name="og",
                                 bufs=1)
                    if ho % 2 == 0:
                        nc.scalar.activation(og[:], acc[:], COPY)
                    else:
                        nc.vector.tensor_copy(og[:], acc[:])
                    oeng = nc.scalar if ho % 2 == 0 else nc.sync
                    oeng.dma_start(
                        out=out_e.ap()[wb, ho * 128:(ho + 1) * 128, wi0:wi0 + IB],
                        in_=og[:])

            for b in range(B):
                # bf16 q/k: halves SBUF traffic on the scores matmuls;
                # numerically validated at ~4.6e-3 rel err vs 2e-2 gate
                qT = st.tile([128, QH, S], BF16, tag="qT")
                kT = st.tile([128, S], BF16, tag="kT")
                vn = st.tile([128, NJT, HD], BF16, tag="vn")
                ccb = st.tile([128, S], F32, tag="cc")
                ssb = st.tile([128, S], F32, tag="ss")
                nc.scalar.dma_start(out=ccb[:], in_=cc_e.ap()[b])
                nc.scalar.dma_start(out=ssb[:], in_=ss_e.ap()[b])

                # interleaved: projection block sb feeds attention block
                # ib == sb immediately (causal: ib needs only j <= (sb+1)*SB)
                for sb in range(NSB):
                    s0 = sb * SB
                    # ---- projection + rotary for seq block sb ----
                    qa = [ps.tile([128, SB], F32, tag="ps", name=f"qa{rc}")
                          for rc in range(QH)]
                    ka = ps.tile([128, SB], F32, tag="ps")
                    vta = ps.tile([128, SB], F32, tag="ps")
                    for hg in range(HC // 4):
                        load_wchunk(hg)
                        xt_g = xp.tile([128, 4, SB], BF16, tag=f"xt{hg % 4}",
                                       bufs=1)
                        eng = nc.sync if hg % 2 == 0 else nc.scalar
                        eng.dma_start(
                            out=xt_g[:],
                            in_=xt_e.ap()[b, hg * 512:(hg + 1) * 512, s0:s0 + SB]
                            .rearrange("(c p) s -> p c s", c=4))
                        for c in range(4):
                            hc = hg * 4 + c
                            st_, sp_ = (hc == 0), (hc == HC - 1)
                            for rc in range(QH):
                                nc.tensor.matmul(qa[rc][:],
                                                 wq_s[:, hc, rc * 128:(rc + 1) * 128],
                                                 xt_g[:, c], start=st_, stop=sp_)
                            nc.tensor.matmul(ka[:], wk_s[:, hc], xt_g[:, c],
                                             start=st_, stop=sp_)
                            nc.tensor.matmul(vta[:], wv_s[:, hc], xt_g[:, c],
                                             start=st_, stop=sp_)
                    load_tail_weights()
                    rotary(ka, s0, ccb, ssb, kT[:, s0:s0 + SB])
                    for rc in range(QH):
                        rotary(qa[rc], s0, ccb, ssb, qT[:, rc, s0:s0 + SB])

                    # previous block's wo matmuls go here: independent PE
                    # work that covers the rotary-chain latency on ACT/DVE
                    # and the vta->vt_s copy before the transposes
                    if pending_wo:
                        emit_wo(*pending_wo.pop())

                    # vT [d, s] -> natural v j-tiles via PE transpose (bf16)
                    vt_s = rp.tile([128, SB], BF16, tag="vts")
                    nc.scalar.activation(vt_s[:], vta[:], COPY)
                    for t in range(SB // 128):
                        tp = ps.tile([128, 128], BF16, tag="ps", name="tp")
                        nc.tensor.transpose(tp[:], vt_s[:, t * 128:(t + 1) * 128],
                                            id_s[:])
                        nc.scalar.activation(vn[:, (SB // 128) * sb + t], tp[:], COPY)

                    if dbg and b == 0 and sb == NSB - 1:
                        nc.sync.dma_start(out=dq_e.ap(), in_=qT[:])
                        nc.sync.dma_start(out=dk_e.ap(), in_=kT[:])
                        nc.sync.dma_start(out=dv_e.ap(), in_=vn[:])

                    # ---- attention for i block ib == sb ----
                    ib = sb
                    i0 = ib * IB
                    njt = (ib + 1) * (IB // 128)
                    at = ap_.tile([128, QH, IB], BF16, tag="at")
                    pvf = []
                    sums = []
                    for h in range(QH):
                        pv = ps.tile([128, IB], F32, tag="ps")
                        psm = sm_p.tile([128, IB], BF16, tag="psm", name=f"psm{h}")
                        pend = []
                        def flush(stop):
                            pjt, ppb = pend.pop(0)
                            nc.tensor.matmul(pv[:], vn[:, pjt], ppb[:],
                                             start=(pjt == 0), stop=stop)
                        for jt in range(njt):
                            sc = ps.tile([128, IB], F32, tag=f"sc{jt % 2}",
                                         name="sc", bufs=1)
                            nc.tensor.matmul(sc[:], kT[:, jt * 128:(jt + 1) * 128],
                                             qT[:, h, i0:i0 + IB],
                                             start=True, stop=True)
                            pb = pp.tile([128, IB], BF16, tag=f"pb{jt % 5}",
                                         bufs=1)
                            nc.scalar.activation(pb[:], sc[:], EXP)
                            kd = jt - ib * 4
                            if kd >= 0:
                                nc.vector.tensor_mul(pb[:], pb[:], mk_s[:, kd])
                            # probs running sum on DVE (replaces the PE
                            # ones-matmul row-sums, ~117us of PE time)
                            if jt == 0:
                                nc.vector.tensor_copy(psm[:], pb[:])
                            else:
                                nc.vector.tensor_add(psm[:], psm[:], pb[:])
                            pend.append((jt, pb))
                            if len(pend) > 2:
                                flush(False)
                        while pend:
                            flush(stop=(len(pend) == 1))
                        # stash PV; normalization deferred to the loop below
                        pvh = sg.tile([128, IB], F32, tag="pvf", name=f"pvf{h}", bufs=4)
                        nc.scalar.activation(pvh[:], pv[:], COPY)
                        pvf.append(pvh)
                        sums.append(psm)
                    # softmax denominators: one ones-matmul row-sum per head
                    # over the DVE-accumulated probs (replaces per-j-tile
                    # row-sum matmuls), then a K=1 matmul broadcast so the
                    # reciprocal runs at full 128-lane width
                    smhs = []
                    for h in range(QH):
                        smm = ps.tile([1, IB], F32, tag="ps", name=f"smm{h}")
                        nc.tensor.matmul(smm[:], ones_b[:], sums[h][:],
                                         start=True, stop=True)
                        smh = sm_p.tile([1, IB], F32R, tag="rc", name=f"sm{h}")
                        nc.vector.tensor_copy(smh[:], smm[:])
                        smhs.append(smh)
                    for h in range(QH):
                        sb_ps = ps.tile([128, IB], F32, tag="ps", name="sb_ps")
                        nc.tensor.matmul(sb_ps[:], ones1[:], smhs[h][:],
                                         start=True, stop=True)
                        rb = sg.tile([128, IB], F32, tag=f"rb{h % 2}", bufs=1)
                        nc.vector.reciprocal_approx_fast(rb[:], sb_ps[:])
                        nc.gpsimd.tensor_mul(at[:, h], pvf[h][:], rb[:])
                    if dbg and b == 0:
                        nc.sync.dma_start(out=da_e.ap()[ib], in_=at[:])
                    pending_wo.append((b, ib, at))

            emit_wo(*pending_wo.pop())

    nc.compile()
    return nc


def _prep(x, freqs_cos, freqs_sin, wq, wk, wv, wo):
    """Shard + pre-transpose inputs for the 8 cores."""
    # rotary pair permutation: within each 32-partition quadrant, x0 of
    # pairs [16q,16q+16) sits in local slots 0..15 and x1 in 16..31.
    perm = np.zeros(HD, dtype=np.int64)
    pair = np.zeros(128, dtype=np.int64)
    sign = np.zeros(128, dtype=np.float32)
    for q in range(4):
        for t in range(16):
            perm[32 * q + t] = 2 * (16 * q + t)
            perm[32 * q + 16 + t] = 2 * (16 * q + t) + 1
            pair[32 * q + t] = 16 * q + t
            pair[32 * q + 16 + t] = 16 * q + t
            sign[32 * q + t] = -1.0
            sign[32 * q + 16 + t] = 1.0

    xt = np.ascontiguousarray(x.transpose(0, 2, 1)).astype(BF16NP)  # [B,H,S]
    cc = np.ascontiguousarray(freqs_cos[:, :, pair].transpose(0, 2, 1)).astype(np.float32)
    ss2 = np.ascontiguousarray((freqs_sin[:, :, pair] * sign).transpose(0, 2, 1)).astype(np.float32)

    # causal masks for the 4 j-tiles inside a diagonal 512-token i block
    masks = np.zeros((4, 128, IB), dtype=BF16NP)
    jj = np.arange(128)[:, None]
    ii = np.arange(IB)[None, :]
    for k in range(4):
        masks[k] = (k * 128 + jj <= ii).astype(BF16NP)

    scale = np.float32(1.0 / np.sqrt(HD))
    in_maps = []
    for c in range(NCORES):
        wq_c = (wq[c * RQ:(c + 1) * RQ] * scale).reshape(QH, HD, H)[:, perm, :]
        wqt = np.ascontiguousarray(wq_c.reshape(RQ, H).T).astype(BF16NP)
        wk_c = wk[c * HD:(c + 1) * HD][perm, :]
        wkt = np.ascontiguousarray(wk_c.T).astype(BF16NP)
        wvt = np.ascontiguousarray(wv[c * HD:(c + 1) * HD].T).astype(BF16NP)
        wot = np.ascontiguousarray(wo[:, c * RQ:(c + 1) * RQ].T).astype(BF16NP)
        in_maps.append({
            "xt": xt, "cc": cc, "ss2": ss2,
            "wqt": wqt, "wkt": wkt, "wvt": wvt, "wot": wot,
            "masks": masks, "ident": np.eye(128, dtype=BF16NP),
        })
    return in_maps


def kernel(x, freqs_cos, freqs_sin, wq, wk, wv, wo):
    global _CACHED, LAST_EXEC_NS
    x = np.asarray(x, dtype=np.float32)
    freqs_cos = np.asarray(freqs_cos, dtype=np.float32)
    freqs_sin = np.asarray(freqs_sin, dtype=np.float32)
    wq = np.asarray(wq, dtype=np.float32)
    wk = np.asarray(wk, dtype=np.float32)
    wv = np.asarray(wv, dtype=np.float32)
    wo = np.asarray(wo, dtype=np.float32)

    if _CACHED is None:
        _CACHED = _build()
    nc = _CACHED

    in_maps = _prep(x, freqs_cos, freqs_sin, wq, wk, wv, wo)
    res = run_bass_kernel_spmd(nc, in_maps, core_ids=list(range(NCORES)))
    LAST_EXEC_NS = res.exec_time_ns

    # unshard: sum the 8 partial wo products, then [B,H,S] -> [B,S,H]
    acc = res.results[0]["out"].astype(np.float64)
    for c in range(1, NCORES):
        acc += res.results[c]["out"]
    return np.ascontiguousarray(acc.transpose(0, 2, 1)).astype(np.float32)



# revision 19
# speedup vs baseline: 1.3160x; 1.0374x over previous
"""Multi-head GQA attention (B=2, S=2048, H=4096, 32 q-heads / 8 kv-heads,
HD=128, rotary, causal) on 8 TRN2 NeuronCores.

Sharding: tensor-parallel over heads, 8-way — core c owns q-heads
[4c, 4c+4) and kv-head c; wq/wk/wv column-sharded, wo row-sharded.  Each
core computes a partial wo product over its head slice for both batches;
the host sums the 8 partials (the TP reduction) and transposes back.

All on-device dataflow is in transposed layout [feature, seq] so no
device-side transposes are needed; the host pre-transposes x and the
weight shards during sharding.  Rotary pairs are laid out so the (x0,x1)
pair swap is a 32-partition-quadrant stream_shuffle on the Vector engine.
"""
import sys

if "/opt/trn_rl_repo" not in sys.path:
    sys.path.insert(0, "/opt/trn_rl_repo")

import numpy as np
import ml_dtypes

from concourse import bacc, tile, mybir, bass_isa
from concourse.bass_utils import run_bass_kernel_spmd

F32 = mybir.dt.float32
F32R = mybir.dt.float32r
BF16 = mybir.dt.bfloat16
EXP = mybir.ActivationFunctionType.Exp
COPY = mybir.ActivationFunctionType.Copy
BF16NP = ml_dtypes.bfloat16

B, S, H = 2, 2048, 4096
NH, NKV, HD = 32, 8, 128
NCORES = 8
QH = NH // NCORES          # 4 q-heads per core
RQ = QH * HD               # 512 q rows per core
SB = 512                   # projection seq block
NSB = S // SB              # 4
IB = 512                   # attention i block
NIB = S // IB              # 4
HC = H // 128              # 32 contraction chunks
NJT = S // 128             # 16 j tiles

# stream_shuffle mask: swap 16-partition halves within each 32-partition quadrant
SHUF = list(range(16, 32)) + list(range(16))

LAST_EXEC_NS = None
_CACHED = None


def _build(dbg=False):
    nc = bacc.Bacc("TRN2", target_bir_lowering=False, debug=False,
                   num_devices=NCORES)

    xt_e = nc.dram_tensor("xt", [B, H, S], BF16, kind="ExternalInput")
    cc_e = nc.dram_tensor("cc", [B, 128, S], F32, kind="ExternalInput")
    ss_e = nc.dram_tensor("ss2", [B, 128, S], F32, kind="ExternalInput")
    wq_e = nc.dram_tensor("wqt", [H, RQ], BF16, kind="ExternalInput")
    wk_e = nc.dram_tensor("wkt", [H, HD], BF16, kind="ExternalInput")
    wv_e = nc.dram_tensor("wvt", [H, HD], BF16, kind="ExternalInput")
    wo_e = nc.dram_tensor("wot", [RQ, H], BF16, kind="ExternalInput")
    mk_e = nc.dram_tensor("masks", [4, 128, IB], BF16, kind="ExternalInput")
    id_e = nc.dram_tensor("ident", [128, 128], BF16, kind="ExternalInput")
    out_e = nc.dram_tensor("out", [B, H, S], BF16, kind="ExternalOutput")
    if dbg:
        dq_e = nc.dram_tensor("dbg_qt", [128, QH, S], BF16, kind="ExternalOutput")
        dk_e = nc.dram_tensor("dbg_kt", [128, S], BF16, kind="ExternalOutput")
        dv_e = nc.dram_tensor("dbg_vn", [128, NJT, HD], BF16, kind="ExternalOutput")
        da_e = nc.dram_tensor("dbg_at", [NIB, 128, QH, IB], BF16, kind="ExternalOutput")

    with tile.TileContext(nc) as tc:
        with (nc.allow_low_precision(reason="bf16/f32r compute by design"),
              tc.tile_pool(name="wpool", bufs=1) as wp,
              tc.tile_pool(name="state", bufs=1) as st,
              tc.tile_pool(name="att", bufs=2) as ap_,
              tc.tile_pool(name="xin", bufs=3) as xp,
              tc.tile_pool(name="probs", bufs=5) as pp,
              tc.tile_pool(name="rot", bufs=2) as rp,
              tc.tile_pool(name="stage", bufs=2) as sg,
              tc.tile_pool(name="small", bufs=4) as sm_p,
              tc.tile_pool(name="ps", bufs=6, space="PSUM") as ps):

            # ---- resident weights ----
            wq_s = wp.tile([128, HC, RQ], BF16)
            wk_s = wp.tile([128, HC, HD], BF16)
            wv_s = wp.tile([128, HC, HD], BF16)
            wo_s = wp.tile([128, QH, H], BF16)
            mk_s = wp.tile([128, 4, IB], BF16)
            id_s = wp.tile([128, 128], BF16)
            ones_b = wp.tile([128, 1], BF16)
            ones1f = wp.tile([1, 128], F32)
            ones1 = wp.tile([1, 128], F32R)
            nc.vector.memset(ones_b[:], 1.0)
            nc.vector.memset(ones1f[:], 1.0)
            nc.vector.tensor_copy(ones1[:], ones1f[:])

            # weight chunks are DMA'd just-in-time, interleaved with the
            # first projection block's x loads so the PE starts immediately
            _wchunk_loaded = [False] * (HC // 4)
            _tail_loaded = [False]

            def load_wchunk(hg):
                # one grouped DMA per 4 contraction chunks
                if _wchunk_loaded[hg]:
                    return
                _wchunk_loaded[hg] = True
                r0, r1 = hg * 512, (hg + 1) * 512
                nc.sync.dma_start(
                    out=wq_s[:, 4 * hg:4 * hg + 4],
                    in_=wq_e.ap()[r0:r1].rearrange("(c p) r -> p c r", c=4))
                nc.sync.dma_start(
                    out=wk_s[:, 4 * hg:4 * hg + 4],
                    in_=wk_e.ap()[r0:r1].rearrange("(c p) r -> p c r", c=4))
                nc.sync.dma_start(
                    out=wv_s[:, 4 * hg:4 * hg + 4],
                    in_=wv_e.ap()[r0:r1].rearrange("(c p) r -> p c r", c=4))

            def load_tail_weights():
                if _tail_loaded[0]:
                    return
                _tail_loaded[0] = True
                nc.sync.dma_start(out=id_s[:], in_=id_e.ap())
                for k in range(4):
                    nc.sync.dma_start(out=mk_s[:, k], in_=mk_e.ap()[k])
                for rc in range(QH):
                    nc.sync.dma_start(out=wo_s[:, rc], in_=wo_e.ap()[rc * 128:(rc + 1) * 128])

            _rot_n = [0]

            def rotary(src_ps, s0, ccb, ssb, dst_ap):
                # dst = src*CC + shuffle(src)*SS2
                r = _rot_n[0] = (_rot_n[0] + 1) % 2
                qs = rp.tile([128, SB], F32, tag=f"qs{r}", bufs=1)
                nc.scalar.activation(qs[:], src_ps[:], COPY)
                qw = rp.tile([128, SB], F32, tag=f"qw{r}", bufs=1)
                nc.vector.stream_shuffle(qw[:], qs[:], SHUF)
                nc.vector.tensor_mul(qs[:], qs[:], ccb[:, s0:s0 + SB])
                nc.vector.tensor_mul(qw[:], qw[:], ssb[:, s0:s0 + SB])
                nc.vector.tensor_add(dst_ap, qs[:], qw[:])

            pending_wo = []

            def emit_wo(wb, wib, wat, ho0, ho1):
                # wo matmuls for output row chunks [ho0, ho1) -- emitted in
                # pieces so they interleave with the attention head loop as
                # PE filler work under the EXP-latency bubbles
                wi0 = wib * IB
                for ho in range(ho0, ho1):
                    acc = ps.tile([128, IB], F32, tag="ps", name="acc")
                    for rc in range(QH):
                        nc.tensor.matmul(acc[:],
                                         wo_s[:, rc, ho * 128:(ho + 1) * 128],
                                         wat[:, rc],
                                         start=(rc == 0), stop=(rc == QH - 1))
                    og = sg.tile([128, IB], BF16, tag=f"og{ho % 3}", name="og",
                                 bufs=1)
                    if ho % 2 == 0:
                        nc.scalar.activation(og[:], acc[:], COPY)
                    else:
                        nc.vector.tensor_copy(og[:], acc[:])
                    oeng = nc.scalar if ho % 2 == 0 else nc.sync
                    oeng.dma_start(
                        out=out_e.ap()[wb, ho * 128:(ho + 1) * 128, wi0:wi0 + IB],
                        in_=og[:])

            for b in range(B):
                # bf16 q/k: halves SBUF traffic on the scores matmuls;
                # numerically validated at ~4.6e-3 rel err vs 2e-2 gate
                qT = st.tile([128, QH, S], BF16, tag="qT")
                kT = st.tile([128, S], BF16, tag="kT")
                vn = st.tile([128, NJT, HD], BF16, tag="vn")
                ccb = st.tile([128, S], F32, tag="cc")
                ssb = st.tile([128, S], F32, tag="ss")
                nc.scalar.dma_start(out=ccb[:], in_=cc_e.ap()[b])
                nc.scalar.dma_start(out=ssb[:], in_=ss_e.ap()[b])

                # interleaved: projection block sb feeds attention block
                # ib == sb immediately (causal: ib needs only j <= (sb+1)*SB)
                for sb in range(NSB):
                    s0 = sb * SB
                    # ---- projection + rotary for seq block sb ----
                    qa = [ps.tile([128, SB], F32, tag="ps", name=f"qa{rc}")
                          for rc in range(QH)]
                    ka = ps.tile([128, SB], F32, tag="ps")
                    vta = ps.tile([128, SB], F32, tag="ps")
                    for hg in range(HC // 4):
                        load_wchunk(hg)
                        xt_g = xp.tile([128, 4, SB], BF16, tag=f"xt{hg % 4}",
                                       bufs=1)
                        eng = nc.sync if hg % 2 == 0 else nc.scalar
                        eng.dma_start(
                            out=xt_g[:],
                            in_=xt_e.ap()[b, hg * 512:(hg + 1) * 512, s0:s0 + SB]
                            .rearrange("(c p) s -> p c s", c=4))
                        for c in range(4):
                            hc = hg * 4 + c
                            st_, sp_ = (hc == 0), (hc == HC - 1)
                            for rc in range(QH):
                                nc.tensor.matmul(qa[rc][:],
                                                 wq_s[:, hc, rc * 128:(rc + 1) * 128],
                                                 xt_g[:, c], start=st_, stop=sp_)
                            nc.tensor.matmul(ka[:], wk_s[:, hc], xt_g[:, c],
                                             start=st_, stop=sp_)
                            nc.tensor.matmul(vta[:], wv_s[:, hc], xt_g[:, c],
                                             start=st_, stop=sp_)
                    load_tail_weights()
                    rotary(ka, s0, ccb, ssb, kT[:, s0:s0 + SB])
                    for rc in range(QH):
                        rotary(qa[rc], s0, ccb, ssb, qT[:, rc, s0:s0 + SB])

                    # first chunk of the previous block's wo matmuls:
                    # independent PE work that covers the rotary-chain
                    # latency on ACT/DVE and the vta->vt_s copies
                    if pending_wo:
                        emit_wo(*pending_wo[0], 0, 8)

                    # vT [d, s] -> natural v j-tiles via PE transpose (bf16),
                    # with the PSUM->SBUF copy chunked so transpose t can
                    # start as soon as its 128-column chunk lands
                    vt_s = rp.tile([128, SB], BF16, tag="vts")
                    for t in range(SB // 128):
                        if t % 2 == 0:
                            nc.scalar.activation(vt_s[:, t * 128:(t + 1) * 128],
                                                 vta[:, t * 128:(t + 1) * 128],
                                                 COPY)
                        else:
                            nc.vector.tensor_copy(vt_s[:, t * 128:(t + 1) * 128],
                                                  vta[:, t * 128:(t + 1) * 128])
                        tp = ps.tile([128, 128], BF16, tag="ps", name="tp")
                        nc.tensor.transpose(tp[:], vt_s[:, t * 128:(t + 1) * 128],
                                            id_s[:])
                        if t % 2 == 0:
                            nc.vector.tensor_copy(vn[:, (SB // 128) * sb + t], tp[:])
                        else:
                            nc.scalar.activation(vn[:, (SB // 128) * sb + t], tp[:],
                                                 COPY)

                    if dbg and b == 0 and sb == NSB - 1:
                        nc.sync.dma_start(out=dq_e.ap(), in_=qT[:])
                        nc.sync.dma_start(out=dk_e.ap(), in_=kT[:])
                        nc.sync.dma_start(out=dv_e.ap(), in_=vn[:])

                    # ---- attention for i block ib == sb ----
                    ib = sb
                    i0 = ib * IB
                    njt = (ib + 1) * (IB // 128)
                    at = ap_.tile([128, QH, IB], BF16, tag="at")
                    pvf = []
                    sums = []
                    for h in range(QH):
                        pv = ps.tile([128, IB], F32, tag="ps")
                        psm = sm_p.tile([128, IB], BF16, tag="psm", name=f"psm{h}")
                        pend = []
                        def flush(stop):
                            pjt, ppb = pend.pop(0)
                            nc.tensor.matmul(pv[:], vn[:, pjt], ppb[:],
                                             start=(pjt == 0), stop=stop)
                        for jt in range(njt):
                            sc = ps.tile([128, IB], F32, tag=f"sc{jt % 2}",
                                         name="sc", bufs=1)
                            nc.tensor.matmul(sc[:], kT[:, jt * 128:(jt + 1) * 128],
                                             qT[:, h, i0:i0 + IB],
                                             start=True, stop=True)
                            pb = pp.tile([128, IB], BF16, tag=f"pb{jt % 5}",
                                         bufs=1)
                            nc.scalar.activation(pb[:], sc[:], EXP)
                            kd = jt - ib * 4
                            if kd >= 0:
                                nc.vector.tensor_mul(pb[:], pb[:], mk_s[:, kd])
                            # probs running sum on DVE (replaces the PE
                            # ones-matmul row-sums, ~117us of PE time)
                            if jt == 0:
                                nc.vector.tensor_copy(psm[:], pb[:])
                            else:
                                nc.vector.tensor_add(psm[:], psm[:], pb[:])
                            pend.append((jt, pb))
                            if len(pend) > 2:
                                flush(False)
                        while pend:
                            flush(stop=(len(pend) == 1))
                        # stash PV; normalization deferred to the loop below
                        pvh = sg.tile([128, IB], F32, tag="pvf", name=f"pvf{h}", bufs=4)
                        nc.scalar.activation(pvh[:], pv[:], COPY)
                        pvf.append(pvh)
                        sums.append(psm)
                        # interleave the previous block's remaining wo chunks
                        # between attention heads: PE filler for EXP bubbles
                        if pending_wo and h < QH - 1:
                            emit_wo(*pending_wo[0], 8 * (h + 1), 8 * (h + 2))
                    # softmax denominators: one ones-matmul row-sum per head
                    # over the DVE-accumulated probs (replaces per-j-tile
                    # row-sum matmuls), then a K=1 matmul broadcast so the
                    # reciprocal runs at full 128-lane width
                    smhs = []
                    for h in range(QH):
                        smm = ps.tile([1, IB], F32, tag="ps", name=f"smm{h}")
                        nc.tensor.matmul(smm[:], ones_b[:], sums[h][:],
                                         start=True, stop=True)
                        smh = sm_p.tile([1, IB], F32R, tag="rc", name=f"sm{h}")
                        nc.vector.tensor_copy(smh[:], smm[:])
                        smhs.append(smh)
                    for h in range(QH):
                        sb_ps = ps.tile([128, IB], F32, tag="ps", name="sb_ps")
                        nc.tensor.matmul(sb_ps[:], ones1[:], smhs[h][:],
                                         start=True, stop=True)
                        rb = sg.tile([128, IB], F32, tag=f"rb{h % 2}", bufs=1)
                        nc.vector.reciprocal_approx_fast(rb[:], sb_ps[:])
                        nc.gpsimd.tensor_mul(at[:, h], pvf[h][:], rb[:])
                    if dbg and b == 0:
                        nc.sync.dma_start(out=da_e.ap()[ib], in_=at[:])
                    if pending_wo:
                        pending_wo.pop()
                    pending_wo.append((b, ib, at))

            emit_wo(*pending_wo.pop(), 0, H // 128)

    nc.compile()
    return nc


def _prep(x, freqs_cos, freqs_sin, wq, wk, wv, wo):
    """Shard + pre-transpose inputs for the 8 cores."""
    # rotary pair permutation: within each 32-partition quadrant, x0 of
    # pairs [16q,16q+16) sits in local slots 0..15 and x1 in 16..31.
    perm = np.zeros(HD, dtype=np.int64)
    pair = np.zeros(128, dtype=np.int64)
    sign = np.zeros(128, dtype=np.float32)
    for q in range(4):
        for t in range(16):
            perm[32 * q + t] = 2 * (16 * q + t)
            perm[32 * q + 16 + t] = 2 * (16 * q + t) + 1
            pair[32 * q + t] = 16 * q + t
            pair[32 * q + 16 + t] = 16 * q + t
            sign[32 * q + t] = -1.0
            sign[32 * q + 16 + t] = 1.0

    xt = np.ascontiguousarray(x.transpose(0, 2, 1)).astype(BF16NP)  # [B,H,S]
    cc = np.ascontiguousarray(freqs_cos[:, :, pair].transpose(0, 2, 1)).astype(np.float32)
    ss2 = np.ascontiguousarray((freqs_sin[:, :, pair] * sign).transpose(0, 2, 1)).astype(np.float32)

    # causal masks for the 4 j-tiles inside a diagonal 512-token i block
    masks = np.zeros((4, 128, IB), dtype=BF16NP)
    jj = np.arange(128)[:, None]
    ii = np.arange(IB)[None, :]
    for k in range(4):
        masks[k] = (k * 128 + jj <= ii).astype(BF16NP)

    scale = np.float32(1.0 / np.sqrt(HD))
    in_maps = []
    for c in range(NCORES):
        wq_c = (wq[c * RQ:(c + 1) * RQ] * scale).reshape(QH, HD, H)[:, perm, :]
        wqt = np.ascontiguousarray(wq_c.reshape(RQ, H).T).astype(BF16NP)
        wk_c = wk[c * HD:(c + 1) * HD][perm, :]
        wkt = np.ascontiguousarray(wk_c.T).astype(BF16NP)
        wvt = np.ascontiguousarray(wv[c * HD:(c + 1) * HD].T).astype(BF16NP)
        wot = np.ascontiguousarray(wo[:, c * RQ:(c + 1) * RQ].T).astype(BF16NP)
        in_maps.append({
            "xt": xt, "cc": cc, "ss2": ss2,
            "wqt": wqt, "wkt": wkt, "wvt": wvt, "wot": wot,
            "masks": masks, "ident": np.eye(128, dtype=BF16NP),
        })
    return in_maps


def kernel(x, freqs_cos, freqs_sin, wq, wk, wv, wo):
    global _CACHED, LAST_EXEC_NS
    x = np.asarray(x, dtype=np.float32)
    freqs_cos = np.asarray(freqs_cos, dtype=np.float32)
    freqs_sin = np.asarray(freqs_sin, dtype=np.float32)
    wq = np.asarray(wq, dtype=np.float32)
    wk = np.asarray(wk, dtype=np.float32)
    wv = np.asarray(wv, dtype=np.float32)
    wo = np.asarray(wo, dtype=np.float32)

    if _CACHED is None:
        _CACHED = _build()
    nc = _CACHED

    in_maps = _prep(x, freqs_cos, freqs_sin, wq, wk, wv, wo)
    res = run_bass_kernel_spmd(nc, in_maps, core_ids=list(range(NCORES)))
    LAST_EXEC_NS = res.exec_time_ns

    # unshard: sum the 8 partial wo products, then [B,H,S] -> [B,S,H]
    acc = res.results[0]["out"].astype(np.float64)
    for c in range(1, NCORES):
        acc += res.results[c]["out"]
    return np.ascontiguousarray(acc.transpose(0, 2, 1)).astype(np.float32)

